# revision 21
# baseline (speedup 1.0000x reference)
"""Binarized VGG-style CNN (CIFAR, batch 256) on 8 TRN2 NeuronCores.

Data-parallel: batch 256 -> 8 x 32. One Bass program, per-core input maps.

Math: for every conv layer 1..6 the network only consumes sign(BN(...)),
and BN is monotone (gamma>0 here), so each layer reduces to
    bits_{l+1} = sign(conv_l(bits_l) + (bias_l - t_l)),  t = m - b/s, s = g/sqrt(v+eps)
with maxpool commuting with sign. All intermediate activations are exactly
+-1 (or 0 on pad border), so conv2..7 run exactly in fp8 (fp32 PSUM
accumulation of integer sums). Only conv1 (real input) is fp32.
"""

import numpy as np

import concourse.bass as bass
import concourse.bacc as bacc
import concourse.tile as tile
import concourse.mybir as mybir
from concourse.bass_utils import run_bass_kernel_spmd

F32 = mybir.dt.float32
FP8 = mybir.dt.float8e4
NP_FP8 = mybir.dt.np(FP8)

N_CORES = 8
B = 32  # images per core
EPS = 1e-5

ALU = mybir.AluOpType
ACTF = mybir.ActivationFunctionType

# layer configs for conv2..conv6:
# (name, IG, OG, Hp_in (padded in spatial), Ho (conv out spatial), pool)
CONV_CFG = {
    2: dict(IG=1, OG=1, Hp=34, Ho=32, pool=True),
    3: dict(IG=1, OG=2, Hp=18, Ho=16, pool=False),
    4: dict(IG=2, OG=2, Hp=18, Ho=16, pool=True),
    5: dict(IG=2, OG=4, Hp=10, Ho=8, pool=False),
    6: dict(IG=4, OG=4, Hp=10, Ho=8, pool=True),
}

_CACHE = {}


def _build(dump=False):
    nc = bacc.Bacc("TRN2", target_bir_lowering=False, debug=False)

    x_d = nc.dram_tensor("x", [B, 3, 32, 32], F32, kind="ExternalInput")
    w1_d = nc.dram_tensor("w1s", [27, 128], F32, kind="ExternalInput")
    be_d = {1: nc.dram_tensor("be1", [128, 1], F32, kind="ExternalInput")}
    w_d = {}
    for l, c in CONV_CFG.items():
        w_d[l] = nc.dram_tensor(
            f"w{l}s", [128, c["IG"], 9, c["OG"], 128], FP8, kind="ExternalInput"
        )
        be_d[l] = nc.dram_tensor(f"be{l}", [128, c["OG"]], F32, kind="ExternalInput")
    w7_d = nc.dram_tensor("w7s", [128, 4, 16, 10], FP8, kind="ExternalInput")
    sf7_d = nc.dram_tensor("sf7", [1, 10], F32, kind="ExternalInput")
    df7_d = nc.dram_tensor("df7", [1, 10], F32, kind="ExternalInput")
    out_d = nc.dram_tensor("out", [B, 10], F32, kind="ExternalOutput")

    with tile.TileContext(nc) as tc:
        with (
            tc.tile_pool(name="wpool", bufs=1) as wpool,
            tc.tile_pool(name="apool", bufs=1) as apool,
            tc.tile_pool(name="xim", bufs=4) as xim,
            tc.tile_pool(name="tpool", bufs=4) as tpool,
            tc.tile_pool(name="spool", bufs=2) as spool,
            tc.tile_pool(name="psum", bufs=6, space="PSUM") as pp,
            tc.tile_pool(name="psum7", bufs=1, space="PSUM") as pp7,
            tc.tile_pool(name="dram", bufs=1, space="DRAM") as dpool,
        ):
            # ---- persistent weight tiles ----
            w1_t = wpool.tile([27, 128], F32, tag="w1")
            nc.sync.dma_start(w1_t[:], w1_d[:])
            w_t, be_t = {}, {}
            for l, c in CONV_CFG.items():
                w_t[l] = wpool.tile([128, c["IG"], 9, c["OG"], 128], FP8, tag=f"w{l}", name=f"w{l}t")
                nc.sync.dma_start(w_t[l][:], w_d[l][:])
                be_t[l] = wpool.tile([128, c["OG"]], F32, tag=f"be{l}", name=f"be{l}t")
                nc.sync.dma_start(be_t[l][:], be_d[l][:])
            be1_t = wpool.tile([128, 1], F32, tag="be1")
            nc.sync.dma_start(be1_t[:], be_d[1][:])
            w7_t = wpool.tile([128, 4, 16, 10], FP8, tag="w7")
            nc.sync.dma_start(w7_t[:], w7_d[:])
            # broadcast [1,10] -> [32,10]
            sf7_t = wpool.tile([B, 10], F32, tag="sf7")
            a = sf7_d[:]
            nc.sync.dma_start(
                sf7_t[:], bass.AP(tensor=a.tensor, offset=a.offset, ap=[[0, B], [1, 10]])
            )
            df7_t = wpool.tile([B, 10], F32, tag="df7")
            a = df7_d[:]
            nc.sync.dma_start(
                df7_t[:], bass.AP(tensor=a.tensor, offset=a.offset, ap=[[0, B], [1, 10]])
            )

            # ---- activation bit-buffers (fp8, zero pad borders) ----
            buf1 = apool.tile([128, B, 34, 34], FP8, tag="buf1")
            buf2 = apool.tile([128, B, 18, 18], FP8, tag="buf2")
            buf3 = apool.tile([128, 2, B, 18, 18], FP8, tag="buf3")
            buf4 = apool.tile([128, 2, B, 10, 10], FP8, tag="buf4")
            buf5 = apool.tile([128, 4, B, 10, 10], FP8, tag="buf5")
            buf6 = apool.tile([128, 4, B, 4, 4], FP8, tag="buf6")

            # zero the pad borders (interior is always overwritten).
            def zero_borders(buf, G, Hp):
                # buf is [128, (G,) B, Hp, Hp]; border rows + border cols.
                for g in range(max(G, 1)):
                    v = buf[:, g] if G else buf[:]
                    vr = v.rearrange("p b h w -> p b h w")
                    # rows 0 and Hp-1 (all cols)
                    ap_rows = bass.AP(
                        tensor=vr.tensor,
                        offset=vr.offset,
                        ap=[vr.ap[0], vr.ap[1], [(Hp - 1) * Hp, 2], [1, Hp]],
                    )
                    nc.gpsimd.memset(ap_rows, 0.0)
                    # cols 0 and Hp-1 (all rows)
                    ap_cols = bass.AP(
                        tensor=vr.tensor,
                        offset=vr.offset,
                        ap=[vr.ap[0], vr.ap[1], [Hp, Hp], [Hp - 1, 2]],
                    )
                    nc.gpsimd.memset(ap_cols, 0.0)

            zero_borders(buf1, 0, 34)
            zero_borders(buf2, 0, 18)
            zero_borders(buf3, 2, 18)
            zero_borders(buf4, 2, 10)
            zero_borders(buf5, 4, 10)

            # ---- stage padded input in DRAM ----
            xpad = dpool.tile([B, 3, 34, 34], F32, tag="xpad")
            zt = wpool.tile([128, 34 * 34], F32, tag="zt")
            nc.vector.memset(zt[:], 0.0)
            xp_flat = xpad[:].rearrange("b c h w -> (b c) (h w)")
            nc.sync.dma_start(xp_flat[0:96, :], zt[:96, :])
            for i in range(B):
                nc.sync.dma_start(xpad[i, :, 1:33, 1:33], x_d[i])

            # ---- conv1: K=27 im2col, fp32 ----
            for i in range(B):
                im = xim.tile([27, 32, 32], F32, tag="im2col")
                for dy in range(3):
                    for c in range(3):
                        src = bass.AP(
                            tensor=xpad[:].tensor,
                            offset=xpad[:].offset + (i * 3 + c) * 34 * 34 + dy * 34,
                            ap=[[1, 3], [34, 32], [1, 32]],
                        )
                        nc.sync.dma_start(im[9 * dy + 3 * c : 9 * dy + 3 * c + 3], src)
                for h in range(2):
                    ps = pp.tile([128, 16, 32], F32, tag="ps")
                    nc.tensor.matmul(ps[:], w1_t[:], im[:, 16 * h : 16 * h + 16, :],
                                     start=True, stop=True)
                    nc.scalar.sign(
                        buf1[:, i, 1 + 16 * h : 17 + 16 * h, 1:33], ps[:], bias=be1_t[:, 0:1]
                    )

            # ---- generic conv layer ----
            def conv_layer(l, bin_, bout, gin, gout):
                c = CONV_CFG[l]
                IG, OG, Hp, Ho, pool = c["IG"], c["OG"], c["Hp"], c["Ho"], c["pool"]
                wt, bet = w_t[l], be_t[l]
                # psum tiling: images (and rows for l=2) per 512-elem tile
                if l == 2:
                    tiles = [(i, h) for i in range(B) for h in range(2)]
                elif Ho == 16:
                    tiles = [(2 * p, None) for p in range(B // 2)]
                else:
                    tiles = [(8 * q, None) for q in range(B // 8)]
                for og in range(OG):
                    for (i0, half) in tiles:
                        if l == 2:
                            ps = pp.tile([128, 16, 32], F32, tag="ps")
                        elif Ho == 16:
                            ps = pp.tile([128, 2, 16, 16], F32, tag="ps")
                        else:
                            ps = pp.tile([128, 8, 8, 8], F32, tag="ps")
                        n_mm = IG * 9
                        k = 0
                        for cg in range(IG):
                            for dy in range(3):
                                for dx in range(3):
                                    if l == 2:
                                        rhs = bin_[:, i0, dy + 16 * half : dy + 16 * half + 16,
                                                   dx : dx + 32]
                                    elif Ho == 16:
                                        src = bin_[:, cg] if gin else bin_[:]
                                        rhs = src[:, i0 : i0 + 2, dy : dy + 16, dx : dx + 16]
                                    else:
                                        src = bin_[:, cg] if gin else bin_[:]
                                        rhs = src[:, i0 : i0 + 8, dy : dy + 8, dx : dx + 8]
                                    nc.tensor.matmul(
                                        ps[:], wt[:, cg, 3 * dy + dx, og, :], rhs,
                                        start=(k == 0), stop=(k == n_mm - 1),
                                    )
                                    k += 1
                        bias = bet[:, og : og + 1]
                        dst_root = bout[:, og] if gout else bout[:]
                        if not pool:
                            # sign straight into padded interior of bout
                            if Ho == 16:
                                dst = dst_root[:, i0 : i0 + 2, 1:17, 1:17]
                            else:
                                dst = dst_root[:, i0 : i0 + 8, 1:9, 1:9]
                            nc.scalar.sign(dst, ps[:], bias=bias)
                        else:
                            # sign first (commutes with maxpool), then 2x2 pool
                            if l == 2:
                                tmp = tpool.tile([128, 16, 32], FP8, tag=f"tmpa{l}")
                                nc.scalar.sign(tmp[:], ps[:], bias=bias)
                                t2 = tpool.tile([128, 16, 16], FP8, tag=f"tmpb{l}")
                                pw = tmp[:].rearrange("p h (w two) -> p h w two", two=2)
                                nc.vector.tensor_max(t2[:], pw[:, :, :, 0], pw[:, :, :, 1])
                                ph = t2[:].rearrange("p (h two) w -> p h two w", two=2)
                                dst = dst_root[:, i0, 1 + 8 * half : 9 + 8 * half, 1:17]
                                nc.vector.tensor_max(dst, ph[:, :, 0, :], ph[:, :, 1, :])
                            elif Ho == 16:
                                tmp = tpool.tile([128, 2, 16, 16], FP8, tag=f"tmpa{l}")
                                nc.scalar.sign(tmp[:], ps[:], bias=bias)
                                t2 = tpool.tile([128, 2, 16, 8], FP8, tag=f"tmpb{l}")
                                pw = tmp[:].rearrange("p b h (w two) -> p b h w two", two=2)
                                nc.vector.tensor_max(t2[:], pw[:, :, :, :, 0], pw[:, :, :, :, 1])
                                ph = t2[:].rearrange("p b (h two) w -> p b h two w", two=2)
                                dst = dst_root[:, i0 : i0 + 2, 1:9, 1:9]
                                nc.vector.tensor_max(dst, ph[:, :, :, 0, :], ph[:, :, :, 1, :])
                            else:
                                tmp = tpool.tile([128, 8, 8, 8], FP8, tag=f"tmpa{l}")
                                nc.scalar.sign(tmp[:], ps[:], bias=bias)
                                t2 = tpool.tile([128, 8, 8, 4], FP8, tag=f"tmpb{l}")
                                pw = tmp[:].rearrange("p b h (w two) -> p b h w two", two=2)
                                nc.vector.tensor_max(t2[:], pw[:, :, :, :, 0], pw[:, :, :, :, 1])
                                ph = t2[:].rearrange("p b (h two) w -> p b h two w", two=2)
                                dst = dst_root[:, i0 : i0 + 8, :, :]
                                nc.vector.tensor_max(dst, ph[:, :, :, 0, :], ph[:, :, :, 1, :])

            conv_layer(2, buf1, buf2, False, False)
            conv_layer(3, buf2, buf3, False, True)
            conv_layer(4, buf3, buf4, True, True)
            conv_layer(5, buf4, buf5, True, True)
            conv_layer(6, buf5, buf6, True, True)

            # ---- conv7 (4x4 VALID -> [B,10]) + BN1d + log_softmax ----
            ps7 = pp7.tile([B, 10], F32, tag="ps7")
            k = 0
            for g in range(4):
                for dy in range(4):
                    for dx in range(4):
                        nc.tensor.matmul(
                            ps7[:], buf6[:, g, :, dy, dx], w7_t[:, g, 4 * dy + dx, :],
                            start=(k == 0), stop=(k == 63),
                        )
                        k += 1
            z = spool.tile([B, 10], F32, tag="z")
            nc.vector.tensor_mul(z[:], ps7[:], sf7_t[:])
            nc.vector.tensor_add(z[:], z[:], df7_t[:])
            nmax = spool.tile([B, 1], F32, tag="nmax")
            nc.vector.tensor_reduce(nmax[:], z[:], axis=mybir.AxisListType.X,
                                    op=ALU.max, negate=True)
            e = spool.tile([B, 10], F32, tag="e")
            se = spool.tile([B, 1], F32, tag="se")
            nc.scalar.activation(e[:], z[:], ACTF.Exp, bias=nmax[:], scale=1.0,
                                 accum_out=se[:])
            lse = spool.tile([B, 1], F32, tag="lse")
            nc.scalar.activation(lse[:], se[:], ACTF.Ln)
            res = spool.tile([B, 10], F32, tag="res")
            nc.vector.tensor_scalar(res[:], z[:], nmax[:], lse[:],
                                    op0=ALU.add, op1=ALU.subtract)
            nc.sync.dma_start(out_d[:], res[:])

            if dump:
                for nm, bt in [("dbg1", buf1), ("dbg2", buf2), ("dbg3", buf3),
                               ("dbg4", buf4), ("dbg5", buf5), ("dbg6", buf6)]:
                    dd = nc.dram_tensor(nm, list(bt.shape), FP8, kind="ExternalOutput")
                    nc.sync.dma_start(dd[:], bt[:])
                d7 = nc.dram_tensor("dbg7", [B, 10], F32, kind="ExternalOutput")
                d7s = spool.tile([B, 10], F32, tag="d7s")
                nc.scalar.copy(d7s[:], ps7[:])
                nc.sync.dma_start(d7[:], d7s[:])

    nc.compile()
    return nc


PM = mybir.MatmulPerfMode

# v2 plane geometry: images packed side-by-side along width, shared separator
# cols (zero), pad rows top/bottom, 16-element guard at both ends.
PLANE = {
    1: dict(Wp=1072, W=32, H=32, stride=33),   # buf1 / L2 input
    2: dict(Wp=560, W=16, H=16, stride=17),    # buf2,3 / L3,L4 input
    3: dict(Wp=304, W=8, H=8, stride=9),       # buf4,5 / L5,L6 input
}
for _v in PLANE.values():
    _v["SZ"] = (_v["H"] + 2) * _v["Wp"] + 32


def _pl_chunks(Wp, Hval):
    """512-chunks over valid rows 1..Hval; returns (abs_lin, n)."""
    total = Hval * Wp
    out, o = [], 0
    while o < total:
        n = min(512, total - o)
        out.append((Wp + o, n))
        o += n
    return out


def _ap(base, off, dims):
    return bass.AP(tensor=base.tensor, offset=base.offset + off, ap=[base.ap[0]] + dims)


def _build_v2(dump=False):
    nc = bacc.Bacc("TRN2", target_bir_lowering=False, debug=False)

    xim_d = nc.dram_tensor("xim", [B, 27, 1024], F32, kind="ExternalInput")
    w1_d = nc.dram_tensor("w1s", [27, 128], F32, kind="ExternalInput")
    be_d = {1: nc.dram_tensor("be1", [128, 1], F32, kind="ExternalInput")}
    w_d = {}
    for l in (2, 3):
        w_d[l] = nc.dram_tensor(f"w{l}p", [128, 3, 3, 128 * CONV_CFG[l]["OG"]], FP8,
                                kind="ExternalInput")
    for l in (4, 5, 6):
        c = CONV_CFG[l]
        w_d[l] = nc.dram_tensor(
            f"w{l}s", [128, c["IG"], 9, c["OG"], 128], FP8, kind="ExternalInput"
        )
    for l in (2, 3, 4, 5, 6):
        be_d[l] = nc.dram_tensor(f"be{l}", [128, CONV_CFG[l]["OG"]], F32,
                                 kind="ExternalInput")
    w7_d = nc.dram_tensor("w7s", [128, 4, 16, 10], FP8, kind="ExternalInput")
    sf7_d = nc.dram_tensor("sf7", [1, 10], F32, kind="ExternalInput")
    df7_d = nc.dram_tensor("df7", [1, 10], F32, kind="ExternalInput")
    out_d = nc.dram_tensor("out", [B, 10], F32, kind="ExternalOutput")

    SZ1, SZ2, SZ3 = PLANE[1]["SZ"], PLANE[2]["SZ"], PLANE[3]["SZ"]

    with tile.TileContext(nc) as tc:
        with (
            tc.tile_pool(name="wpool", bufs=1) as wpool,
            tc.tile_pool(name="apool", bufs=1) as apool,
            tc.tile_pool(name="xim", bufs=4) as xim,
            tc.tile_pool(name="tpool", bufs=4) as tpool,
            tc.tile_pool(name="spool", bufs=2) as spool,
            tc.tile_pool(name="psum", bufs=6, space="PSUM") as pp,
            tc.tile_pool(name="psum7", bufs=1, space="PSUM") as pp7,
            tc.tile_pool(name="dram", bufs=1, space="DRAM") as dpool,
        ):
            w1_t = wpool.tile([27, 128], F32, tag="w1")
            nc.sync.dma_start(w1_t[:], w1_d[:])
            be1_t = wpool.tile([128, 1], F32, tag="be1")
            nc.sync.dma_start(be1_t[:], be_d[1][:])
            w_t, be_t = {}, {}

            def load_weights():
                for l in (2, 3):
                    w_t[l] = wpool.tile([128, 3, 3, 128 * CONV_CFG[l]["OG"]], FP8,
                                        tag=f"w{l}", name=f"w{l}t")
                    nc.gpsimd.dma_start(w_t[l][:], w_d[l][:])
                for l in (4, 5, 6):
                    c = CONV_CFG[l]
                    w_t[l] = wpool.tile([128, c["IG"], 9, c["OG"], 128], FP8,
                                        tag=f"w{l}", name=f"w{l}t")
                    nc.gpsimd.dma_start(w_t[l][:], w_d[l][:])
                for l in (2, 3, 4, 5, 6):
                    be_t[l] = wpool.tile([128, CONV_CFG[l]["OG"]], F32, tag=f"be{l}",
                                         name=f"be{l}t")
                    nc.gpsimd.dma_start(be_t[l][:], be_d[l][:])
            w7_t = wpool.tile([128, 4, 16, 10], FP8, tag="w7")
            nc.sync.dma_start(w7_t[:], w7_d[:])
            sf7_t = wpool.tile([B, 10], F32, tag="sf7")
            a = sf7_d[:]
            nc.sync.dma_start(
                sf7_t[:], bass.AP(tensor=a.tensor, offset=a.offset, ap=[[0, B], [1, 10]])
            )
            df7_t = wpool.tile([B, 10], F32, tag="df7")
            a = df7_d[:]
            nc.sync.dma_start(
                df7_t[:], bass.AP(tensor=a.tensor, offset=a.offset, ap=[[0, B], [1, 10]])
            )

            # activation planes
            P1 = apool.tile([128, SZ1], FP8, tag="P1")
            P2 = apool.tile([128, SZ2], FP8, tag="P2")
            P3 = apool.tile([128, 2, SZ2], FP8, tag="P3")
            P4 = apool.tile([128, 2, SZ3], FP8, tag="P4")
            P5 = apool.tile([128, 4, SZ3], FP8, tag="P5")
            buf6 = apool.tile([128, 4, 4, 128], FP8, tag="buf6")
            scr2 = apool.tile([128, 16 * 1072], FP8, tag="scr2")
            scr4 = apool.tile([128, 16 * 560], FP8, tag="scr4")
            scr6 = apool.tile([128, 8 * 304], FP8, tag="scr6")

            def pad_memset(Pt, goff, pl):
                Wp, H, st = pl["Wp"], pl["H"], pl["stride"]
                base = Pt[:]
                # separator cols (incl left pad col), all rows
                nc.gpsimd.memset(
                    _ap(base, goff + 16, [[Wp, H + 2], [st, B + 1]]), 0.0)
                # top/bottom pad rows (separate: ISA AP steps are 16-bit)
                nc.gpsimd.memset(_ap(base, goff + 16, [[1, Wp]]), 0.0)
                nc.gpsimd.memset(
                    _ap(base, goff + 16 + (H + 1) * Wp, [[1, Wp]]), 0.0)
                # unused tail cols + head/tail guards (never valid-read, but
                # keep them finite/initialized)
                used = st * B + 1
                if Wp > used:
                    nc.gpsimd.memset(
                        _ap(base, goff + 16 + used, [[Wp, H + 2], [1, Wp - used]]), 0.0)
                nc.gpsimd.memset(_ap(base, goff, [[1, 16]]), 0.0)
                nc.gpsimd.memset(
                    _ap(base, goff + 16 + (H + 2) * Wp, [[1, 16]]), 0.0)

            # ---- conv1 from host-prepared im2col (8-image blocks) ----
            pad_memset(P1, 0, PLANE[1])
            for i in range(B):
                im = xim.tile([27, 32, 32], F32, tag="im2col")
                eng = nc.sync if i % 2 == 0 else nc.gpsimd
                eng.dma_start(im[:], xim_d[i].rearrange("k (h w) -> k h w", w=32))
                for h in range(2):
                    ps = pp.tile([128, 16, 32], F32, tag="ps")
                    nc.tensor.matmul(ps[:], w1_t[:], im[:, 16 * h : 16 * h + 16, :],
                                     start=True, stop=True)
                    nc.scalar.sign(
                        _ap(P1[:], 16 + (1 + 16 * h) * 1072 + 33 * i + 1,
                            [[1072, 16], [1, 32]]),
                        ps[:], bias=be1_t[:, 0:1],
                    )
            load_weights()

            # ---- dy-paired layer (IG=1): L2 (pool, banded) and L3 ----
            def mm_dy_pairs(Pin, wt, og, o, n, Wp, ps):
                # 4 DoubleRow pairs + 1 single:
                #   3 dy-pairs (dy 0,1 per dx; pair step Wp)
                #   1 dx-pair at dy=2 (dx 0,1; pair step 1)
                #   single (dy=2, dx=2)
                k, last = 0, 4
                osl = slice(og * 128, (og + 1) * 128)
                for dx in range(3):
                    rhs = _ap(Pin[:], 16 + o - Wp + dx - 1, [[Wp, 2], [1, n]])
                    nc.tensor.matmul(ps[:], wt[:, dx, 0:2, osl], rhs,
                                     start=(k == 0), stop=(k == last),
                                     perf_mode=PM.DoubleRow)
                    k += 1
                rhs = _ap(Pin[:], 16 + o + Wp - 1, [[1, 2], [1, n]])
                nc.tensor.matmul(ps[:], wt[:, 0:2, 2, osl], rhs,
                                 start=(k == 0), stop=(k == last),
                                 perf_mode=PM.DoubleRow)
                k += 1
                rhs = _ap(Pin[:], 16 + o + Wp + 1, [[1, n]])
                nc.tensor.matmul(ps[:], wt[:, 2, 2, osl], rhs,
                                 start=(k == 0), stop=(k == last))
                k += 1

            def mm_cg_pairs(Pin, wt, og, o, n, Wp, SZg, IG, ps):
                k, last = 0, IG // 2 * 9 - 1
                for pr in range(IG // 2):
                    for dy in range(3):
                        for dx in range(3):
                            rhs = _ap(Pin[:], 2 * pr * SZg + 16 + o + (dy - 1) * Wp + dx - 1,
                                      [[SZg, 2], [1, n]])
                            nc.tensor.matmul(
                                ps[:], wt[:, 2 * pr : 2 * pr + 2, 3 * dy + dx, og, :],
                                rhs, start=(k == 0), stop=(k == last),
                                perf_mode=PM.DoubleRow)
                            k += 1

            def pool_row(scr, loc_row, Wp_in, st_in, W_half, dst_ap, tag):
                # 2x2 maxpool of scratch rows loc_row, loc_row+1 -> dst_ap
                m1 = tpool.tile([128, B, W_half], FP8, tag=f"m1{tag}")
                m2 = tpool.tile([128, B, W_half], FP8, tag=f"m2{tag}")
                for j, m in ((0, m1), (1, m2)):
                    off = (loc_row + j) * Wp_in + 1
                    nc.vector.tensor_max(
                        m[:],
                        _ap(scr[:], off, [[st_in, B], [2, W_half]]),
                        _ap(scr[:], off + 1, [[st_in, B], [2, W_half]]),
                    )
                nc.vector.tensor_max(dst_ap, m1[:], m2[:])

            # L2: 2 bands of 16 rows
            for b in range(2):
                band0 = (1 + 16 * b) * 1072
                total = 16 * 1072
                o = 0
                while o < total:
                    n = min(512, total - o)
                    ps = pp.tile([128, 512], F32, tag="ps")
                    mm_dy_pairs(P1, w_t[2], 0, band0 + o, n, 1072, ps[:, :n])
                    nc.scalar.sign(scr2[:, o : o + n], ps[:, :n], bias=be_t[2][:, 0:1])
                    o += n
                for R in range(1 + 8 * b, 9 + 8 * b):
                    loc = 2 * (R - 1) - 16 * b
                    pool_row(scr2, loc, 1072, 33, 16,
                             _ap(P2[:], 16 + R * 560 + 1, [[17, 32], [1, 16]]), "a")
            pad_memset(P2, 0, PLANE[2])

            # L3
            for og in range(2):
                for (o, n) in _pl_chunks(560, 16):
                    ps = pp.tile([128, 512], F32, tag="ps")
                    mm_dy_pairs(P2, w_t[3], og, o, n, 560, ps[:, :n])
                    nc.scalar.sign(P3[:, og, 16 + o : 16 + o + n], ps[:, :n],
                                   bias=be_t[3][:, og : og + 1])
            for og in range(2):
                pad_memset(P3, og * SZ2, PLANE[2])

            # L4 (cg pairs, pool)
            for og in range(2):
                for (o, n) in _pl_chunks(560, 16):
                    ps = pp.tile([128, 512], F32, tag="ps")
                    mm_cg_pairs(P3, w_t[4], og, o, n, 560, SZ2, 2, ps[:, :n])
                    nc.scalar.sign(scr4[:, o - 560 : o - 560 + n], ps[:, :n],
                                   bias=be_t[4][:, og : og + 1])
                for R in range(1, 9):
                    pool_row(scr4, 2 * (R - 1), 560, 17, 8,
                             _ap(P4[:], og * SZ3 + 16 + R * 304 + 1, [[9, 32], [1, 8]]),
                             "b")
            for og in range(2):
                pad_memset(P4, og * SZ3, PLANE[3])

            # L5
            for og in range(4):
                for (o, n) in _pl_chunks(304, 8):
                    ps = pp.tile([128, 512], F32, tag="ps")
                    mm_cg_pairs(P4, w_t[5], og, o, n, 304, SZ3, 2, ps[:, :n])
                    nc.scalar.sign(P5[:, og, 16 + o : 16 + o + n], ps[:, :n],
                                   bias=be_t[5][:, og : og + 1])
            for og in range(4):
                pad_memset(P5, og * SZ3, PLANE[3])

            # L6 (cg pairs x2, pool) with conv7 group og interleaved
            ps7 = pp7.tile([B, 10], F32, tag="ps7")
            for og in range(4):
                for (o, n) in _pl_chunks(304, 8):
                    ps = pp.tile([128, 512], F32, tag="ps")
                    mm_cg_pairs(P5, w_t[6], og, o, n, 304, SZ3, 4, ps[:, :n])
                    nc.scalar.sign(scr6[:, o - 304 : o - 304 + n], ps[:, :n],
                                   bias=be_t[6][:, og : og + 1])
                for R in range(1, 5):
                    dst = buf6[:, og, R - 1].rearrange("p (i w) -> p i w", w=4)
                    pool_row(scr6, 2 * (R - 1), 304, 9, 4, dst, "c")
                for dy in range(4):
                    for dx in range(4):
                        lhsT = buf6[:, og, dy].rearrange("p (i w) -> p i w", w=4)[:, :, dx]
                        nc.tensor.matmul(ps7[:], lhsT, w7_t[:, og, 4 * dy + dx, :],
                                         start=(og == 0 and dy == 0 and dx == 0),
                                         stop=(og == 3 and dy == 3 and dx == 3))

            # ---- BN1d + log_softmax ----
            z = spool.tile([B, 10], F32, tag="z")
            nc.vector.tensor_mul(z[:], ps7[:], sf7_t[:])
            nc.vector.tensor_add(z[:], z[:], df7_t[:])
            nmax = spool.tile([B, 1], F32, tag="nmax")
            nc.vector.tensor_reduce(nmax[:], z[:], axis=mybir.AxisListType.X,
                                    op=ALU.max, negate=True)
            e = spool.tile([B, 10], F32, tag="e")
            se = spool.tile([B, 1], F32, tag="se")
            nc.scalar.activation(e[:], z[:], ACTF.Exp, bias=nmax[:], scale=1.0,
                                 accum_out=se[:])
            lse = spool.tile([B, 1], F32, tag="lse")
            nc.scalar.activation(lse[:], se[:], ACTF.Ln)
            res = spool.tile([B, 10], F32, tag="res")
            nc.vector.tensor_scalar(res[:], z[:], nmax[:], lse[:],
                                    op0=ALU.add, op1=ALU.subtract)
            nc.sync.dma_start(out_d[:], res[:])

            if dump:
                for nm, bt in [("dbgP1", P1), ("dbgP2", P2), ("dbgP3", P3),
                               ("dbgP4", P4), ("dbgP5", P5), ("dbg6", buf6)]:
                    dd = nc.dram_tensor(nm, list(bt.shape), FP8, kind="ExternalOutput")
                    nc.sync.dma_start(dd[:], bt[:])
                d7 = nc.dram_tensor("dbg7", [B, 10], F32, kind="ExternalOutput")
                d7s = spool.tile([B, 10], F32, tag="d7s")
                nc.scalar.copy(d7s[:], ps7[:])
                nc.sync.dma_start(d7[:], d7s[:])

    nc.compile()
    return nc


def _prep_consts(inp):
    """Host-side weight preprocessing -> dict of device input arrays."""
    out = {}
    # device im2col partition order is k = dy*9 + c*3 + dx
    out["w1s"] = np.ascontiguousarray(
        np.sign(inp["w1"]).transpose(2, 1, 3, 0).reshape(27, 128)
    ).astype(np.float32)
    for l, c in CONV_CFG.items():
        IG, OG = c["IG"], c["OG"]
        ws = np.sign(inp[f"w{l}"]).astype(np.float32)  # [cout, cin, 3, 3]
        ws = ws.transpose(1, 2, 3, 0).reshape(IG, 128, 9, OG, 128)
        out[f"w{l}s"] = np.ascontiguousarray(ws.transpose(1, 0, 2, 3, 4)).astype(NP_FP8)
    for l in (2, 3):
        # v2 dy-pair layout: [128(cin), dx, dy, cout]
        ws = np.sign(inp[f"w{l}"]).astype(np.float32)
        out[f"w{l}p"] = np.ascontiguousarray(ws.transpose(1, 3, 2, 0)).astype(NP_FP8)
    for l in range(1, 7):
        g = inp[f"bn{l}_g"].astype(np.float64)
        b = inp[f"bn{l}_b"].astype(np.float64)
        m = inp[f"bn{l}_m"].astype(np.float64)
        v = inp[f"bn{l}_v"].astype(np.float64)
        s = g / np.sqrt(v + EPS)
        t = m - b / s
        be = inp[f"b{l}"].astype(np.float64) - t
        C = be.shape[0]
        OG = C // 128
        out[f"be{l}"] = np.ascontiguousarray(
            be.reshape(OG, 128).T if OG > 1 else be.reshape(128, 1)
        ).astype(np.float32)
    ws7 = np.sign(inp["w7"]).astype(np.float32)  # [10, 512, 4, 4]
    ws7 = ws7.transpose(1, 2, 3, 0).reshape(4, 128, 16, 10)
    out["w7s"] = np.ascontiguousarray(ws7.transpose(1, 0, 2, 3)).astype(NP_FP8)
    sf = inp["bnf_g"].astype(np.float64) / np.sqrt(inp["bnf_v"].astype(np.float64) + EPS)
    df = (inp["b7"].astype(np.float64) - inp["bnf_m"].astype(np.float64)) * sf + inp[
        "bnf_b"
    ].astype(np.float64)
    out["sf7"] = sf.reshape(1, 10).astype(np.float32)
    out["df7"] = df.reshape(1, 10).astype(np.float32)
    return out


def _prep_x_im2col(x):
    """[b,3,32,32] -> [b,27,1024] zero-padded im2col, k = dy*9 + c*3 + dx."""
    b = x.shape[0]
    xp = np.zeros((b, 3, 34, 34), np.float32)
    xp[:, :, 1:33, 1:33] = x
    xim = np.empty((b, 27, 32, 32), np.float32)
    for dy in range(3):
        for c in range(3):
            for dx in range(3):
                xim[:, dy * 9 + c * 3 + dx] = xp[:, c, dy : dy + 32, dx : dx + 32]
    return np.ascontiguousarray(xim.reshape(b, 27, 1024))


def make_in_maps(inputs, version=2):
    consts = _prep_consts(inputs)
    x = np.asarray(inputs["x"], dtype=np.float32)
    in_maps = []
    for c in range(N_CORES):
        m = dict(consts)
        shard = x[c * B : (c + 1) * B]
        m["x"] = np.ascontiguousarray(shard)
        m["xim"] = _prep_x_im2col(shard)
        in_maps.append(m)
    return in_maps


def kernel(**inputs) -> np.ndarray:
    inputs = {k: np.asarray(v) for k, v in inputs.items()}
    if "nc" not in _CACHE:
        _CACHE["nc"] = _build_v2()
    nc = _CACHE["nc"]
    in_maps = make_in_maps(inputs)
    res = run_bass_kernel_spmd(nc, in_maps, list(range(N_CORES)))
    return np.concatenate([r["out"] for r in res.results], axis=0)
